# revision 17
# baseline (speedup 1.0000x reference)
import numpy as np

import concourse.bacc as bacc
import concourse.tile as tile
import concourse.mybir as mybir
from concourse.bass_utils import run_bass_kernel_spmd

B, D, G, GS = 262144, 512, 4, 4
NO = G + G * GS
NCORES = 8
BC = B // NCORES
P = 128
S = BC // P
CH = D // P
NT = 8
NBLK = S // NT
GPB = NT // 2

F16 = mybir.dt.float16
F32 = mybir.dt.float32
I32 = mybir.dt.int32
AX = mybir.AxisListType
OP = mybir.AluOpType

_cached_nc = None


def _build(xs_bufs=4, pxt_bufs=4, pl_bufs=4, dve_mod=(4, 1), smear=2):
    nc = bacc.Bacc("TRN2", target_bir_lowering=False, num_devices=NCORES)
    x = nc.dram_tensor("x", [BC, D], F16, kind="ExternalInput")
    whl = nc.dram_tensor("whl", [D, 2 * NO], F16, kind="ExternalInput")
    bias = nc.dram_tensor("bias", [2 * NO], F16, kind="ExternalInput")
    idx_o = nc.dram_tensor("idx_o", [BC, 2], I32, kind="ExternalOutput")
    w_o = nc.dram_tensor("w_o", [BC, 2], F32, kind="ExternalOutput")

    x_r = x.ap().rearrange("(p s) d -> p s d", p=P)
    idx_r = idx_o.ap().rearrange("(p s) k -> p s k", p=P)
    w_r = w_o.ap().rearrange("(p s) k -> p s k", p=P)

    with tile.TileContext(nc) as tc:
        with (
            tc.tile_pool(name="singles", bufs=1) as singles,
            tc.tile_pool(name="xs_pool", bufs=xs_bufs) as xs_pool,
            tc.tile_pool(name="xt_pool", bufs=6) as xt_pool,
            tc.tile_pool(name="big", bufs=1) as big,
            tc.tile_pool(name="post", bufs=2) as post,
            tc.tile_pool(name="pxt_pool", bufs=pxt_bufs, space="PSUM") as pxt_pool,
            tc.tile_pool(name="pl_pool", bufs=pl_bufs, space="PSUM") as pl_pool,
        ):
            ident = singles.tile([P, P], F16)
            nc.vector.memset(ident, 1.0)
            nc.gpsimd.affine_select(
                ident, ident, pattern=[[-1, P]], base=0, channel_multiplier=1,
                compare_op=OP.is_equal, fill=0.0)
            wt_sb = singles.tile([P, CH, 2 * NO], F16)
            nc.sync.dma_start(
                out=wt_sb, in_=whl.ap().rearrange("(c p) j -> p c j", p=P))
            bias_sb = singles.tile([1, 2 * NO], F16)
            nc.sync.dma_start(out=bias_sb, in_=bias.ap().unsqueeze(0))
            ones_sb = singles.tile([1, P], F16)
            nc.vector.memset(ones_sb, 1.0)
            kconst = singles.tile([P, GS], F32)
            k4const = singles.tile([P, G], F32)
            for j in range(GS):
                nc.vector.memset(kconst[:, j:j + 1], float(j))
                nc.vector.memset(k4const[:, j:j + 1], float(j * GS))

            L = big.tile([P, S, NO], F32)
            iout = big.tile([P, S, 2], I32)
            wout_t = big.tile([P, S, 2], F32)

            def pp_ops(s0, s1, tail=False):
                SH = s1 - s0
                Gv = L[:, s0:s1, 0:G]
                INv = L[:, s0:s1, G:NO].rearrange("p s (g k) -> p s g k", g=G)
                kb = kconst.unsqueeze(1).broadcast_to([P, SH, GS])
                k4b = k4const.unsqueeze(1).broadcast_to([P, SH, G])

                def bcast(t):
                    return t[:, :].unsqueeze(2).broadcast_to([P, SH, 4])

                tt_eng = nc.vector if tail else nc.gpsimd
                v = {}

                def alloc(name, shape, dt=F32):
                    v[name] = post.tile(shape, dt, name=name)
                    return v[name]

                ops = [
                    lambda: nc.vector.tensor_reduce(
                        alloc("gmax", [P, SH]), Gv, axis=AX.X, op=OP.max),
                    lambda: nc.vector.tensor_tensor(
                        alloc("eqg", [P, SH, G]), Gv, bcast(v["gmax"]),
                        op=OP.is_equal),
                    lambda: nc.vector.tensor_tensor(
                        alloc("tmp", [P, SH, GS, G]).rearrange(
                            "p s k g -> p s g k"),
                        v["eqg"].unsqueeze(3).broadcast_to([P, SH, G, GS]),
                        INv, op=OP.mult),
                    lambda: nc.vector.tensor_reduce(
                        alloc("sel", [P, SH, GS]), v["tmp"], axis=AX.X,
                        op=OP.add),
                    lambda: nc.vector.tensor_reduce(
                        alloc("s12", [P, SH, 2])[:, :, 0], v["sel"],
                        axis=AX.X, op=OP.max),
                    lambda: nc.vector.tensor_tensor(
                        alloc("eq1", [P, SH, GS]), v["sel"],
                        bcast(v["s12"][:, :, 0]), op=OP.is_equal),
                    lambda: nc.vector.scalar_tensor_tensor(
                        alloc("pm", [P, SH, GS]), v["eq1"], -1e30, v["sel"],
                        op0=OP.mult, op1=OP.add),
                    lambda: nc.vector.tensor_reduce(
                        v["s12"][:, :, 1], v["pm"], axis=AX.X, op=OP.max),
                    lambda: nc.vector.tensor_tensor(
                        alloc("eq2", [P, SH, GS]), v["pm"],
                        bcast(v["s12"][:, :, 1]), op=OP.is_equal),
                    lambda: nc.scalar.activation(
                        alloc("e", [P, SH, GS]), v["sel"],
                        func=mybir.ActivationFunctionType.Exp),
                    lambda: nc.vector.tensor_reduce(
                        alloc("ssum", [P, SH]), v["e"], axis=AX.X, op=OP.add),
                    lambda: nc.vector.reciprocal(
                        alloc("rcp", [P, SH]), v["ssum"]),
                    lambda: nc.scalar.activation(
                        alloc("e12", [P, SH, 2]), v["s12"],
                        func=mybir.ActivationFunctionType.Exp),
                    lambda: nc.vector.tensor_tensor(
                        wout_t[:, s0:s1, :], v["e12"],
                        v["rcp"][:, :].unsqueeze(2).broadcast_to([P, SH, 2]),
                        op=OP.mult),
                    lambda: tt_eng.tensor_tensor(
                        alloc("tk", [P, SH, GS]), v["eq1"], kb, op=OP.mult),
                    lambda: nc.vector.tensor_reduce(
                        alloc("i1", [P, SH]), v["tk"], axis=AX.X, op=OP.add),
                    lambda: tt_eng.tensor_tensor(
                        alloc("tk2", [P, SH, GS]), v["eq2"], kb, op=OP.mult),
                    lambda: nc.vector.tensor_reduce(
                        alloc("i2", [P, SH]), v["tk2"], axis=AX.X, op=OP.add),
                    lambda: tt_eng.tensor_tensor(
                        alloc("tg", [P, SH, G]), v["eqg"], k4b, op=OP.mult),
                    lambda: nc.vector.tensor_reduce(
                        alloc("g4", [P, SH]), v["tg"], axis=AX.X, op=OP.add),
                    lambda: nc.vector.tensor_tensor(
                        iout[:, s0:s1, 0], v["g4"], v["i1"], op=OP.add),
                    lambda: nc.vector.tensor_tensor(
                        iout[:, s0:s1, 1], v["g4"], v["i2"], op=OP.add),
                ]
                return ops

            pending = []
            NCH = 8
            SC = S // NCH
            PPC = SC // 2
            for blk in range(NBLK):
                xs = xs_pool.tile([P, NT, D], F16)
                nc.sync.dma_start(out=xs, in_=x_r[:, NT * blk:NT * (blk + 1), :])
                for gp in range(GPB):
                    g = GPB * blk + gp
                    pxt = pxt_pool.tile([P, 2, CH, P], F16)
                    for ti in range(2):
                        for c in range(CH):
                            nc.tensor.transpose(
                                pxt[:, ti, c, :],
                                xs[:, 2 * gp + ti, c * P:(c + 1) * P], ident)
                    xt = xt_pool.tile([P, 2, CH, P], F16)
                    if g % dve_mod[0] < dve_mod[1] or g >= 120:
                        nc.vector.tensor_copy(xt, pxt)
                    else:
                        nc.scalar.copy(
                            xt[:, :, :, :].rearrange(
                                "p a c n -> p (a c n)").bitcast(F32),
                            pxt[:, :, :, :].rearrange(
                                "p a c n -> p (a c n)").bitcast(F32))
                    pl = pl_pool.tile([P, 2, NO], F32)
                    for ti in range(2):
                        nc.tensor.matmul(pl[:, ti, :], ones_sb,
                                         bias_sb[:, 0:NO], start=True,
                                         stop=False)
                        for c in range(CH):
                            nc.tensor.matmul(
                                pl[:, ti, :], xt[:, ti, c, :],
                                wt_sb[:, c, 0:NO],
                                start=False, stop=False)
                            nc.tensor.matmul(
                                pl[:, ti, :], xt[:, ti, c, :],
                                wt_sb[:, c, NO:2 * NO],
                                start=False, stop=(c == CH - 1))
                    nc.vector.tensor_copy(L[:, 2 * g:2 * g + 2, :], pl)
                    for _ in range(smear):
                        if pending:
                            pending.pop(0)()
                    if (g + 1) % PPC == 0:
                        k = (g + 1) // PPC - 1
                        if k < NCH - 1:
                            pending.extend(pp_ops(SC * k, SC * (k + 1)))
                        else:
                            pending.extend(pp_ops(SC * k, SC * k + SC // 2,
                                                  tail=True))
                            pending.extend(pp_ops(SC * k + SC // 2,
                                                  SC * (k + 1), tail=True))
                        if k == 3:
                            def _ship_first_half():
                                h = S // 2
                                nc.scalar.dma_start(out=idx_r[:, 0:h, :],
                                                    in_=iout[:, 0:h, :])
                                nc.scalar.dma_start(out=w_r[:, 0:h, :],
                                                    in_=wout_t[:, 0:h, :])
                            pending.append(_ship_first_half)
            for op in pending:
                op()

            half = S // 2
            nc.scalar.dma_start(out=idx_r[:, half:S, :],
                                in_=iout[:, half:S, :])
            nc.scalar.dma_start(out=w_r[:, half:S, :],
                                in_=wout_t[:, half:S, :])
    nc.finalize()
    return nc


def _get_nc():
    global _cached_nc
    if _cached_nc is None:
        _cached_nc = _build()
    return _cached_nc


def kernel(routing_features, group_w, group_b, in_w, in_b, experts_table,
           trace=False):
    x = np.asarray(routing_features, np.float32).astype(np.float16)
    gw = np.asarray(group_w, np.float32)
    gb = np.asarray(group_b, np.float32)
    iw = np.asarray(in_w, np.float32).reshape(G * GS, D)
    ib = np.asarray(in_b, np.float32).reshape(G * GS)
    table = np.asarray(experts_table, np.int32).reshape(-1)

    wt = np.concatenate([gw, iw], 0).T.astype(np.float32)
    w_hi = wt.astype(np.float16)
    w_lo = (wt - w_hi.astype(np.float32)).astype(np.float16)
    whl = np.ascontiguousarray(np.concatenate([w_hi, w_lo], 1))
    b20 = np.concatenate([gb, ib], 0).astype(np.float16)
    bias = np.concatenate([b20, np.zeros(NO, np.float16)])

    shards = np.ascontiguousarray(x.reshape(NCORES, BC, D))
    in_maps = [{"x": shards[c], "whl": whl, "bias": bias}
               for c in range(NCORES)]
    try:
        res = run_bass_kernel_spmd(_get_nc(), in_maps,
                                   core_ids=list(range(NCORES)), trace=trace)
    except (ImportError, ModuleNotFoundError):
        res = run_bass_kernel_spmd(_get_nc(), in_maps,
                                   core_ids=list(range(NCORES)), trace=False)
    idx = np.concatenate([res.results[c]["idx_o"] for c in range(NCORES)], 0)
    w = np.concatenate([res.results[c]["w_o"] for c in range(NCORES)], 0)
    expert_indices = table[idx]
    if trace:
        kernel.last_exec_time_ns = res.exec_time_ns
        if kernel.last_exec_time_ns is None:
            try:
                from concourse.timeline_sim import TimelineSim
                kernel.last_exec_time_ns = int(TimelineSim(_get_nc()).simulate())
                kernel.time_source = "cost-model timeline sim"
            except Exception:
                pass
        else:
            kernel.time_source = "ntff"
    return expert_indices, w


# revision 21
# speedup vs baseline: 1.1033x; 1.1033x over previous
import numpy as np

import concourse.bacc as bacc
import concourse.tile as tile
import concourse.mybir as mybir
from concourse.bass_utils import run_bass_kernel_spmd

B, D, G, GS = 262144, 512, 4, 4
NO = G + G * GS
NCORES = 8
BC = B // NCORES
P = 128
S = BC // P
CH = D // P
NT = 8
NBLK = S // NT
GPB = NT // 2

F16 = mybir.dt.float16
F32 = mybir.dt.float32
I32 = mybir.dt.int32
AX = mybir.AxisListType
OP = mybir.AluOpType

_cached_nc = None


def _build(xs_bufs=8, pxt_bufs=4, pl_bufs=4, dve_mod=(4, 1), smear=2, F16C=True, L_alt=False, lowpri=False, nt=2, dve_end=128, end_alt=True, alt_start=80):
    nc = bacc.Bacc("TRN2", target_bir_lowering=False, num_devices=NCORES)
    x = nc.dram_tensor("x", [BC, D], F16, kind="ExternalInput")
    whl = nc.dram_tensor("whl", [D, 2 * NO], F16, kind="ExternalInput")
    bias = nc.dram_tensor("bias", [2 * NO], F16, kind="ExternalInput")
    idx_o = nc.dram_tensor("idx_o", [BC, 2], I32, kind="ExternalOutput")
    w_o = nc.dram_tensor("w_o", [BC, 2], F32, kind="ExternalOutput")

    x_r = x.ap().rearrange("(p s) d -> p s d", p=P)
    idx_r = idx_o.ap().rearrange("(p s) k -> p s k", p=P)
    w_r = w_o.ap().rearrange("(p s) k -> p s k", p=P)

    with tile.TileContext(nc) as tc:
        with (
            tc.tile_pool(name="singles", bufs=1) as singles,
            tc.tile_pool(name="xs_pool", bufs=xs_bufs) as xs_pool,
            tc.tile_pool(name="xt_pool", bufs=6) as xt_pool,
            tc.tile_pool(name="big", bufs=1) as big,
            tc.tile_pool(name="post", bufs=2) as post,
            tc.tile_pool(name="pxt_pool", bufs=pxt_bufs, space="PSUM") as pxt_pool,
            tc.tile_pool(name="pl_pool", bufs=pl_bufs, space="PSUM") as pl_pool,
        ):
            ident = singles.tile([P, P], F16)
            nc.vector.memset(ident, 1.0)
            nc.gpsimd.affine_select(
                ident, ident, pattern=[[-1, P]], base=0, channel_multiplier=1,
                compare_op=OP.is_equal, fill=0.0)
            wt_sb = singles.tile([P, CH, 2 * NO], F16)
            nc.sync.dma_start(
                out=wt_sb, in_=whl.ap().rearrange("(c p) j -> p c j", p=P))
            bias_sb = singles.tile([1, 2 * NO], F16)
            nc.sync.dma_start(out=bias_sb, in_=bias.ap().unsqueeze(0))
            ones_sb = singles.tile([1, P], F16)
            nc.vector.memset(ones_sb, 1.0)
            kconst = singles.tile([P, GS], F32)
            k4const = singles.tile([P, G], F32)
            for j in range(GS):
                nc.vector.memset(kconst[:, j:j + 1], float(j))
                nc.vector.memset(k4const[:, j:j + 1], float(j * GS))

            L = big.tile([P, S, NO], F32)
            iout = big.tile([P, S, 2], I32)
            wout_t = big.tile([P, S, 2], F32)

            def pp_ops(s0, s1, tail=False):
                SH = s1 - s0
                Gv = L[:, s0:s1, 0:G]
                INv = L[:, s0:s1, G:NO].rearrange("p s (g k) -> p s g k", g=G)
                kb = kconst.unsqueeze(1).broadcast_to([P, SH, GS])
                k4b = k4const.unsqueeze(1).broadcast_to([P, SH, G])

                def bcast(t):
                    return t[:, :].unsqueeze(2).broadcast_to([P, SH, 4])

                tt_eng = nc.vector if tail else nc.gpsimd
                v = {}

                def alloc(name, shape, dt=F32):
                    v[name] = post.tile(shape, dt, name=name)
                    return v[name]

                ops = [
                    lambda: nc.vector.tensor_reduce(
                        alloc("gmax", [P, SH]), Gv, axis=AX.X, op=OP.max),
                    lambda: nc.vector.tensor_tensor(
                        alloc("eqg", [P, SH, G]), Gv, bcast(v["gmax"]),
                        op=OP.is_equal),
                    lambda: tt_eng.tensor_tensor(
                        alloc("tmp", [P, SH, GS, G]).rearrange(
                            "p s k g -> p s g k"),
                        v["eqg"].unsqueeze(3).broadcast_to([P, SH, G, GS]),
                        INv, op=OP.mult),
                    lambda: nc.vector.tensor_reduce(
                        alloc("sel", [P, SH, GS]), v["tmp"], axis=AX.X,
                        op=OP.add),
                    lambda: nc.vector.tensor_reduce(
                        alloc("s12", [P, SH, 2])[:, :, 0], v["sel"],
                        axis=AX.X, op=OP.max),
                    lambda: nc.vector.tensor_tensor(
                        alloc("eq1", [P, SH, GS]), v["sel"],
                        bcast(v["s12"][:, :, 0]), op=OP.is_equal),
                    lambda: nc.vector.scalar_tensor_tensor(
                        alloc("pm", [P, SH, GS]), v["eq1"], -1e30, v["sel"],
                        op0=OP.mult, op1=OP.add),
                    lambda: nc.vector.tensor_reduce(
                        v["s12"][:, :, 1], v["pm"], axis=AX.X, op=OP.max),
                    lambda: nc.vector.tensor_tensor(
                        alloc("eq2", [P, SH, GS]), v["pm"],
                        bcast(v["s12"][:, :, 1]), op=OP.is_equal),
                    lambda: nc.scalar.activation(
                        alloc("e", [P, SH, GS]), v["sel"],
                        func=mybir.ActivationFunctionType.Exp),
                    lambda: nc.vector.tensor_reduce(
                        alloc("ssum", [P, SH]), v["e"], axis=AX.X, op=OP.add),
                    lambda: nc.vector.reciprocal(
                        alloc("rcp", [P, SH]), v["ssum"]),
                    lambda: nc.scalar.activation(
                        alloc("e12", [P, SH, 2]), v["s12"],
                        func=mybir.ActivationFunctionType.Exp),
                    lambda: nc.vector.tensor_tensor(
                        wout_t[:, s0:s1, :], v["e12"],
                        v["rcp"][:, :].unsqueeze(2).broadcast_to([P, SH, 2]),
                        op=OP.mult),
                    lambda: tt_eng.tensor_tensor(
                        alloc("tk", [P, SH, GS]), v["eq1"], kb, op=OP.mult),
                    lambda: nc.vector.tensor_reduce(
                        alloc("i1", [P, SH]), v["tk"], axis=AX.X, op=OP.add),
                    lambda: tt_eng.tensor_tensor(
                        alloc("tk2", [P, SH, GS]), v["eq2"], kb, op=OP.mult),
                    lambda: nc.vector.tensor_reduce(
                        alloc("i2", [P, SH]), v["tk2"], axis=AX.X, op=OP.add),
                    lambda: tt_eng.tensor_tensor(
                        alloc("tg", [P, SH, G]), v["eqg"], k4b, op=OP.mult),
                    lambda: nc.vector.tensor_reduce(
                        alloc("g4", [P, SH]), v["tg"], axis=AX.X, op=OP.add),
                    lambda: nc.vector.tensor_tensor(
                        iout[:, s0:s1, 0], v["g4"], v["i1"], op=OP.add),
                    lambda: nc.vector.tensor_tensor(
                        iout[:, s0:s1, 1], v["g4"], v["i2"], op=OP.add),
                ]
                return ops

            pending = []
            NCH = 8
            SC = S // NCH
            PPC = SC // 2
            nblk = S // nt
            gpb = nt // 2
            for blk in range(nblk):
                xs = xs_pool.tile([P, nt, D], F16)
                nc.sync.dma_start(out=xs, in_=x_r[:, nt * blk:nt * (blk + 1), :])
                for gp in range(gpb):
                    g = gpb * blk + gp
                    pxt = pxt_pool.tile([P, 2, CH, P], F16)
                    for ti in range(2):
                        for c in range(CH):
                            nc.tensor.transpose(
                                pxt[:, ti, c, :],
                                xs[:, 2 * gp + ti, c * P:(c + 1) * P], ident)
                    xt = xt_pool.tile([P, 2, CH, P], F16)
                    if (g % dve_mod[0] < dve_mod[1] or g >= dve_end
                            or (end_alt and g >= alt_start and g % 2 == 0)):
                        nc.vector.tensor_copy(xt, pxt)
                    else:
                        nc.scalar.copy(xt, pxt) if F16C else nc.scalar.copy(
                            xt[:, :, :, :].bitcast(F32),
                            pxt[:, :, :, :].bitcast(F32))
                    pl = pl_pool.tile([P, 2, NO], F32)
                    for ti in range(2):
                        nc.tensor.matmul(pl[:, ti, :], ones_sb,
                                         bias_sb[:, 0:NO], start=True,
                                         stop=False)
                        for c in range(CH):
                            nc.tensor.matmul(
                                pl[:, ti, :], xt[:, ti, c, :],
                                wt_sb[:, c, 0:NO],
                                start=False, stop=False)
                            nc.tensor.matmul(
                                pl[:, ti, :], xt[:, ti, c, :],
                                wt_sb[:, c, NO:2 * NO],
                                start=False, stop=(c == CH - 1))
                    if g % 2 == 0 or not L_alt:
                        nc.vector.tensor_copy(L[:, 2 * g:2 * g + 2, :], pl)
                    else:
                        nc.scalar.copy(L[:, 2 * g:2 * g + 2, :], pl)
                    for _ in range(smear):
                        if pending:
                            op = pending.pop(0)
                            if lowpri:
                                sv = tc.cur_priority
                                tc.cur_priority = sv + 200000
                                op()
                                tc.cur_priority = sv + 1
                            else:
                                op()
                    if (g + 1) % PPC == 0:
                        k = (g + 1) // PPC - 1
                        if k < NCH - 1:
                            pending.extend(pp_ops(SC * k, SC * (k + 1)))
                        else:
                            pending.extend(pp_ops(SC * k, SC * k + SC // 2,
                                                  tail=True))
                            pending.extend(pp_ops(SC * k + SC // 2,
                                                  SC * (k + 1), tail=True))
                        if k == 3:
                            def _ship_first_half():
                                h = S // 2
                                nc.scalar.dma_start(out=idx_r[:, 0:h, :],
                                                    in_=iout[:, 0:h, :])
                                nc.scalar.dma_start(out=w_r[:, 0:h, :],
                                                    in_=wout_t[:, 0:h, :])
                            pending.append(_ship_first_half)
            for op in pending:
                op()

            half = S // 2
            nc.scalar.dma_start(out=idx_r[:, half:S, :],
                                in_=iout[:, half:S, :])
            nc.scalar.dma_start(out=w_r[:, half:S, :],
                                in_=wout_t[:, half:S, :])
    nc.finalize()
    return nc


def _get_nc():
    global _cached_nc
    if _cached_nc is None:
        _cached_nc = _build()
    return _cached_nc


def kernel(routing_features, group_w, group_b, in_w, in_b, experts_table,
           trace=False):
    x = np.asarray(routing_features, np.float32).astype(np.float16)
    gw = np.asarray(group_w, np.float32)
    gb = np.asarray(group_b, np.float32)
    iw = np.asarray(in_w, np.float32).reshape(G * GS, D)
    ib = np.asarray(in_b, np.float32).reshape(G * GS)
    table = np.asarray(experts_table, np.int32).reshape(-1)

    wt = np.concatenate([gw, iw], 0).T.astype(np.float32)
    w_hi = wt.astype(np.float16)
    w_lo = (wt - w_hi.astype(np.float32)).astype(np.float16)
    whl = np.ascontiguousarray(np.concatenate([w_hi, w_lo], 1))
    b20 = np.concatenate([gb, ib], 0).astype(np.float16)
    bias = np.concatenate([b20, np.zeros(NO, np.float16)])

    shards = np.ascontiguousarray(x.reshape(NCORES, BC, D))
    in_maps = [{"x": shards[c], "whl": whl, "bias": bias}
               for c in range(NCORES)]
    try:
        res = run_bass_kernel_spmd(_get_nc(), in_maps,
                                   core_ids=list(range(NCORES)), trace=trace)
    except (ImportError, ModuleNotFoundError):
        res = run_bass_kernel_spmd(_get_nc(), in_maps,
                                   core_ids=list(range(NCORES)), trace=False)
    idx = np.concatenate([res.results[c]["idx_o"] for c in range(NCORES)], 0)
    w = np.concatenate([res.results[c]["w_o"] for c in range(NCORES)], 0)
    expert_indices = table[idx]
    if trace:
        kernel.last_exec_time_ns = res.exec_time_ns
        if kernel.last_exec_time_ns is None:
            try:
                from concourse.timeline_sim import TimelineSim
                kernel.last_exec_time_ns = int(TimelineSim(_get_nc()).simulate())
                kernel.time_source = "cost-model timeline sim"
            except Exception:
                pass
        else:
            kernel.time_source = "ntff"
    return expert_indices, w


# revision 27
# speedup vs baseline: 1.1067x; 1.0031x over previous
import numpy as np

import concourse.bacc as bacc
import concourse.tile as tile
import concourse.mybir as mybir
from concourse.bass_utils import run_bass_kernel_spmd

B, D, G, GS = 262144, 512, 4, 4
NO = G + G * GS
NCORES = 8
BC = B // NCORES
P = 128
S = BC // P
CH = D // P
NT = 8
NBLK = S // NT
GPB = NT // 2

F16 = mybir.dt.float16
F32 = mybir.dt.float32
I32 = mybir.dt.int32
AX = mybir.AxisListType
OP = mybir.AluOpType

_cached_nc = None


def _build(xs_bufs=8, pxt_bufs=4, pl_bufs=4, dve_mod=(4, 1), smear=2, F16C=True, L_alt=False, lowpri=False, nt=2, dve_end=128, end_alt=True, alt_start=80, tail_dve=True, ship4=True, last4=False):
    nc = bacc.Bacc("TRN2", target_bir_lowering=False, num_devices=NCORES)
    x = nc.dram_tensor("x", [BC, D], F16, kind="ExternalInput")
    whl = nc.dram_tensor("whl", [D, 2 * NO], F16, kind="ExternalInput")
    bias = nc.dram_tensor("bias", [2 * NO], F16, kind="ExternalInput")
    idx_o = nc.dram_tensor("idx_o", [BC, 2], I32, kind="ExternalOutput")
    w_o = nc.dram_tensor("w_o", [BC, 2], F32, kind="ExternalOutput")

    x_r = x.ap().rearrange("(p s) d -> p s d", p=P)
    idx_r = idx_o.ap().rearrange("(p s) k -> p s k", p=P)
    w_r = w_o.ap().rearrange("(p s) k -> p s k", p=P)

    with tile.TileContext(nc) as tc:
        with (
            tc.tile_pool(name="singles", bufs=1) as singles,
            tc.tile_pool(name="xs_pool", bufs=xs_bufs) as xs_pool,
            tc.tile_pool(name="xt_pool", bufs=6) as xt_pool,
            tc.tile_pool(name="big", bufs=1) as big,
            tc.tile_pool(name="post", bufs=2) as post,
            tc.tile_pool(name="pxt_pool", bufs=pxt_bufs, space="PSUM") as pxt_pool,
            tc.tile_pool(name="pl_pool", bufs=pl_bufs, space="PSUM") as pl_pool,
        ):
            ident = singles.tile([P, P], F16)
            nc.vector.memset(ident, 1.0)
            nc.gpsimd.affine_select(
                ident, ident, pattern=[[-1, P]], base=0, channel_multiplier=1,
                compare_op=OP.is_equal, fill=0.0)
            wt_sb = singles.tile([P, CH, 2 * NO], F16)
            bias_sb = singles.tile([1, 2 * NO], F16)

            nc.sync.dma_start(
                out=wt_sb, in_=whl.ap().rearrange("(c p) j -> p c j", p=P))
            nc.sync.dma_start(out=bias_sb, in_=bias.ap().unsqueeze(0))
            ones_sb = singles.tile([1, P], F16)
            nc.vector.memset(ones_sb, 1.0)
            kconst = singles.tile([P, GS], F32)
            k4const = singles.tile([P, G], F32)
            for j in range(GS):
                nc.vector.memset(kconst[:, j:j + 1], float(j))
                nc.vector.memset(k4const[:, j:j + 1], float(j * GS))

            L = big.tile([P, S, NO], F32)
            iout = big.tile([P, S, 2], I32)
            wout_t = big.tile([P, S, 2], F32)

            def pp_ops(s0, s1, tail=False):
                SH = s1 - s0
                Gv = L[:, s0:s1, 0:G]
                INv = L[:, s0:s1, G:NO].rearrange("p s (g k) -> p s g k", g=G)
                kb = kconst.unsqueeze(1).broadcast_to([P, SH, GS])
                k4b = k4const.unsqueeze(1).broadcast_to([P, SH, G])

                def bcast(t):
                    return t[:, :].unsqueeze(2).broadcast_to([P, SH, 4])

                tt_eng = nc.vector if tail else nc.gpsimd
                v = {}

                def alloc(name, shape, dt=F32):
                    v[name] = post.tile(shape, dt, name=name)
                    return v[name]

                ops = [
                    lambda: nc.vector.tensor_reduce(
                        alloc("gmax", [P, SH]), Gv, axis=AX.X, op=OP.max),
                    lambda: nc.vector.tensor_tensor(
                        alloc("eqg", [P, SH, G]), Gv, bcast(v["gmax"]),
                        op=OP.is_equal),
                    lambda: tt_eng.tensor_tensor(
                        alloc("tmp", [P, SH, GS, G]).rearrange(
                            "p s k g -> p s g k"),
                        v["eqg"].unsqueeze(3).broadcast_to([P, SH, G, GS]),
                        INv, op=OP.mult),
                    lambda: nc.vector.tensor_reduce(
                        alloc("sel", [P, SH, GS]), v["tmp"], axis=AX.X,
                        op=OP.add),
                    lambda: nc.vector.tensor_reduce(
                        alloc("s12", [P, SH, 2])[:, :, 0], v["sel"],
                        axis=AX.X, op=OP.max),
                    lambda: nc.vector.tensor_tensor(
                        alloc("eq1", [P, SH, GS]), v["sel"],
                        bcast(v["s12"][:, :, 0]), op=OP.is_equal),
                    lambda: nc.vector.scalar_tensor_tensor(
                        alloc("pm", [P, SH, GS]), v["eq1"], -1e30, v["sel"],
                        op0=OP.mult, op1=OP.add),
                    lambda: nc.vector.tensor_reduce(
                        v["s12"][:, :, 1], v["pm"], axis=AX.X, op=OP.max),
                    lambda: nc.vector.tensor_tensor(
                        alloc("eq2", [P, SH, GS]), v["pm"],
                        bcast(v["s12"][:, :, 1]), op=OP.is_equal),
                    lambda: nc.scalar.activation(
                        alloc("e", [P, SH, GS]), v["sel"],
                        func=mybir.ActivationFunctionType.Exp),
                    lambda: nc.vector.tensor_reduce(
                        alloc("ssum", [P, SH]), v["e"], axis=AX.X, op=OP.add),
                    lambda: nc.vector.reciprocal(
                        alloc("rcp", [P, SH]), v["ssum"]),
                    lambda: nc.scalar.activation(
                        alloc("e12", [P, SH, 2]), v["s12"],
                        func=mybir.ActivationFunctionType.Exp),
                    lambda: nc.vector.tensor_tensor(
                        wout_t[:, s0:s1, :], v["e12"],
                        v["rcp"][:, :].unsqueeze(2).broadcast_to([P, SH, 2]),
                        op=OP.mult),
                    lambda: tt_eng.tensor_tensor(
                        alloc("tk", [P, SH, GS]), v["eq1"], kb, op=OP.mult),
                    lambda: nc.vector.tensor_reduce(
                        alloc("i1", [P, SH]), v["tk"], axis=AX.X, op=OP.add),
                    lambda: tt_eng.tensor_tensor(
                        alloc("tk2", [P, SH, GS]), v["eq2"], kb, op=OP.mult),
                    lambda: nc.vector.tensor_reduce(
                        alloc("i2", [P, SH]), v["tk2"], axis=AX.X, op=OP.add),
                    lambda: tt_eng.tensor_tensor(
                        alloc("tg", [P, SH, G]), v["eqg"], k4b, op=OP.mult),
                    lambda: nc.vector.tensor_reduce(
                        alloc("g4", [P, SH]), v["tg"], axis=AX.X, op=OP.add),
                    lambda: nc.vector.tensor_tensor(
                        iout[:, s0:s1, 0], v["g4"], v["i1"], op=OP.add),
                    lambda: nc.vector.tensor_tensor(
                        iout[:, s0:s1, 1], v["g4"], v["i2"], op=OP.add),
                ]
                return ops

            pending = []
            NCH = 8
            SC = S // NCH
            PPC = SC // 2
            nblk = S // nt
            gpb = nt // 2
            for blk in range(nblk):
                xs = xs_pool.tile([P, nt, D], F16)
                nc.sync.dma_start(out=xs, in_=x_r[:, nt * blk:nt * (blk + 1), :])
                for gp in range(gpb):
                    g = gpb * blk + gp
                    pxt = pxt_pool.tile([P, 2, CH, P], F16)
                    for ti in range(2):
                        for c in range(CH):
                            nc.tensor.transpose(
                                pxt[:, ti, c, :],
                                xs[:, 2 * gp + ti, c * P:(c + 1) * P], ident)
                    xt = xt_pool.tile([P, 2, CH, P], F16)
                    if (g % dve_mod[0] < dve_mod[1] or g >= dve_end
                            or (end_alt and g >= alt_start and g % 2 == 0)):
                        nc.vector.tensor_copy(xt, pxt)
                    else:
                        nc.scalar.copy(xt, pxt) if F16C else nc.scalar.copy(
                            xt[:, :, :, :].bitcast(F32),
                            pxt[:, :, :, :].bitcast(F32))
                    pl = pl_pool.tile([P, 2, NO], F32)
                    for ti in range(2):
                        nc.tensor.matmul(pl[:, ti, :], ones_sb,
                                         bias_sb[:, 0:NO], start=True,
                                         stop=False)
                        for c in range(CH):
                            nc.tensor.matmul(
                                pl[:, ti, :], xt[:, ti, c, :],
                                wt_sb[:, c, 0:NO],
                                start=False, stop=False)
                            nc.tensor.matmul(
                                pl[:, ti, :], xt[:, ti, c, :],
                                wt_sb[:, c, NO:2 * NO],
                                start=False, stop=(c == CH - 1))
                    if g % 2 == 0 or not L_alt:
                        nc.vector.tensor_copy(L[:, 2 * g:2 * g + 2, :], pl)
                    else:
                        nc.scalar.copy(L[:, 2 * g:2 * g + 2, :], pl)
                    for _ in range(smear):
                        if pending:
                            op = pending.pop(0)
                            if lowpri:
                                sv = tc.cur_priority
                                tc.cur_priority = sv + 200000
                                op()
                                tc.cur_priority = sv + 1
                            else:
                                op()
                    if (g + 1) % PPC == 0:
                        k = (g + 1) // PPC - 1
                        if k < NCH - 1:
                            pending.extend(pp_ops(SC * k, SC * (k + 1)))
                        else:
                            pending.extend(pp_ops(SC * k, SC * k + SC // 2,
                                                  tail=tail_dve))
                            pending.extend(pp_ops(SC * k + SC // 2,
                                                  SC * (k + 1), tail=tail_dve))
                        if ship4 and k == 5:
                            def _ship_q3():
                                nc.scalar.dma_start(
                                    out=idx_r[:, 128:192, :],
                                    in_=iout[:, 128:192, :])
                                nc.scalar.dma_start(
                                    out=w_r[:, 128:192, :],
                                    in_=wout_t[:, 128:192, :])
                            pending.append(_ship_q3)
                        if k == 3:
                            def _ship_first_half():
                                h = S // 2
                                nc.scalar.dma_start(out=idx_r[:, 0:h, :],
                                                    in_=iout[:, 0:h, :])
                                nc.scalar.dma_start(out=w_r[:, 0:h, :],
                                                    in_=wout_t[:, 0:h, :])
                            pending.append(_ship_first_half)
            for op in pending:
                op()

            lo = 192 if ship4 else S // 2
            nc.scalar.dma_start(out=idx_r[:, lo:S, :],
                                in_=iout[:, lo:S, :])
            nc.scalar.dma_start(out=w_r[:, lo:S, :],
                                in_=wout_t[:, lo:S, :])
    nc.finalize()
    return nc


def _get_nc():
    global _cached_nc
    if _cached_nc is None:
        _cached_nc = _build()
    return _cached_nc


def kernel(routing_features, group_w, group_b, in_w, in_b, experts_table,
           trace=False):
    x = np.asarray(routing_features, np.float32).astype(np.float16)
    gw = np.asarray(group_w, np.float32)
    gb = np.asarray(group_b, np.float32)
    iw = np.asarray(in_w, np.float32).reshape(G * GS, D)
    ib = np.asarray(in_b, np.float32).reshape(G * GS)
    table = np.asarray(experts_table, np.int32).reshape(-1)

    wt = np.concatenate([gw, iw], 0).T.astype(np.float32)
    w_hi = wt.astype(np.float16)
    w_lo = (wt - w_hi.astype(np.float32)).astype(np.float16)
    whl = np.ascontiguousarray(np.concatenate([w_hi, w_lo], 1))
    b20 = np.concatenate([gb, ib], 0).astype(np.float16)
    bias = np.concatenate([b20, np.zeros(NO, np.float16)])

    shards = np.ascontiguousarray(x.reshape(NCORES, BC, D))
    in_maps = [{"x": shards[c], "whl": whl, "bias": bias}
               for c in range(NCORES)]
    try:
        res = run_bass_kernel_spmd(_get_nc(), in_maps,
                                   core_ids=list(range(NCORES)), trace=trace)
    except (ImportError, ModuleNotFoundError):
        res = run_bass_kernel_spmd(_get_nc(), in_maps,
                                   core_ids=list(range(NCORES)), trace=False)
    idx = np.concatenate([res.results[c]["idx_o"] for c in range(NCORES)], 0)
    w = np.concatenate([res.results[c]["w_o"] for c in range(NCORES)], 0)
    expert_indices = table[idx]
    if trace:
        kernel.last_exec_time_ns = res.exec_time_ns
        if kernel.last_exec_time_ns is None:
            try:
                from concourse.timeline_sim import TimelineSim
                kernel.last_exec_time_ns = int(TimelineSim(_get_nc()).simulate())
                kernel.time_source = "cost-model timeline sim"
            except Exception:
                pass
        else:
            kernel.time_source = "ntff"
    return expert_indices, w
